# revision 1
# baseline (speedup 1.0000x reference)
"""Trainium2 Bass kernel: CMAFM fusion (segment min/max stats -> attention
MLPs -> gated 2-layer MLP over voxels), data-parallel over the batch axis.

Sharding: batch b -> NeuronCore b (batch_idx is sorted, B == n_cores == 8).
Each core computes its own batch's feature min/max stats locally, runs the
tiny attention MLPs on-device, folds the per-feature gating into the first
fused-MLP weight matrix, and runs the big MLP over its voxels. No
collectives are needed: every voxel's gating row is core-local by
construction. Shards are padded to a common S_pad with replicated real rows
(min/max invariant); padded outputs are dropped on unshard.
"""

import os
import sys

import numpy as np

for _p in ("/opt/trn_rl_repo",):
    if os.path.isdir(_p) and _p not in sys.path:
        sys.path.append(_p)

B = 8
L = 128
C = 128
OUT = 256
CA = 512
H = 170
VT = 512  # voxels per processing tile

# Precision knobs (module-level so a harness can flip them before first call).
MM_DT = "bf16"   # "f32" | "f32r" | "bf16" - dtype of the big-MLP matmuls
TR_F32R = False  # use float32r for PE transposes
S_RES_CAP = 18944  # voxels (f32) of transposed input kept resident in SBUF
STATS_MODE = "reduce"  # "ttr_pair" | "ttr_contig" | "reduce" (ttr crashes TRN2 HW here)

FMAX = 3.4028234663852886e38

_cache = {}


def _build(S_pad, S_res, mm_dt, tr_f32r, stats_mode="reduce", reps=1):
    from contextlib import ExitStack

    import concourse.bacc as bacc
    import concourse.mybir as mybir
    import concourse.tile as tile
    from concourse import masks

    f32 = mybir.dt.float32
    f32r = mybir.dt.float32r
    bf16 = mybir.dt.bfloat16
    Alu = mybir.AluOpType
    Act = mybir.ActivationFunctionType
    mdt = bf16 if mm_dt == "bf16" else f32  # storage dtype of matmul operands

    n_tiles = S_pad // VT
    n_res = S_res // VT

    nc = bacc.Bacc("TRN2", target_bir_lowering=False, debug=False, num_devices=B)
    lidar = nc.dram_tensor("lidar", [S_pad, L], f32, kind="ExternalInput").ap()
    cam = nc.dram_tensor("cam", [S_pad, C], f32, kind="ExternalInput").ap()
    wl1 = nc.dram_tensor("W_l1", [CA, H], f32, kind="ExternalInput").ap()
    wl2 = nc.dram_tensor("W_l2", [H, L], f32, kind="ExternalInput").ap()
    wc1 = nc.dram_tensor("W_c1", [CA, H], f32, kind="ExternalInput").ap()
    wc2 = nc.dram_tensor("W_c2", [H, C], f32, kind="ExternalInput").ap()
    wf1 = nc.dram_tensor("W_f1", [2 * L, OUT], f32, kind="ExternalInput").ap()
    wf2 = nc.dram_tensor("W_f2", [OUT, OUT], f32, kind="ExternalInput").ap()
    out = nc.dram_tensor("out", [S_pad, OUT], f32, kind="ExternalOutput").ap()

    def mmv(ap):
        return ap.bitcast(f32r) if mm_dt == "f32r" else ap

    def trv(ap):
        return ap.bitcast(f32r) if tr_f32r else ap

    with tile.TileContext(nc) as tc, ExitStack() as ctx:
        wpool = ctx.enter_context(tc.tile_pool(name="weights", bufs=1))
        respool = ctx.enter_context(tc.tile_pool(name="res", bufs=1))
        statpool = ctx.enter_context(tc.tile_pool(name="stat", bufs=1))
        natpool = ctx.enter_context(tc.tile_pool(name="nat", bufs=4))
        gaccpool = ctx.enter_context(tc.tile_pool(name="gacc", bufs=2))
        xtpool = ctx.enter_context(tc.tile_pool(name="xt", bufs=2))
        junkpool = ctx.enter_context(tc.tile_pool(name="junk", bufs=2))
        pstr = ctx.enter_context(tc.tile_pool(name="pstr", bufs=2, space="PSUM"))

        ident = wpool.tile([128, 128], f32)
        masks.make_identity(nc, ident[:])

        wf1_s = wpool.tile([128, 2, OUT], f32)
        nc.sync.dma_start(wf1_s[:], wf1.rearrange("(a p) o -> p a o", p=128))
        wf2_s = wpool.tile([128, 2, OUT], mdt)
        if mm_dt != "f32":
            wf2_stage = wpool.tile([128, 2, OUT], f32)
            nc.sync.dma_start(
                wf2_stage[:], wf2.rearrange("(a p) o -> p a o", p=128)
            )
            nc.vector.tensor_copy(mmv(wf2_s[:]), wf2_stage[:])
        else:
            nc.sync.dma_start(wf2_s[:], wf2.rearrange("(a p) o -> p a o", p=128))
        w1e_s = wpool.tile([128, 2, OUT], mdt)
        wl1_s = wpool.tile([128, 4, H], f32)
        nc.sync.dma_start(wl1_s[:], wl1.rearrange("(a p) h -> p a h", p=128))
        wc1_s = wpool.tile([128, 4, H], f32)
        nc.sync.dma_start(wc1_s[:], wc1.rearrange("(a p) h -> p a h", p=128))
        wl2a_s = wpool.tile([128, L], f32)
        nc.sync.dma_start(wl2a_s[:], wl2[0:128, :])
        wl2b_s = wpool.tile([H - 128, L], f32)
        nc.sync.dma_start(wl2b_s[:], wl2[128:H, :])
        wc2a_s = wpool.tile([128, C], f32)
        nc.sync.dma_start(wc2a_s[:], wc2[0:128, :])
        wc2b_s = wpool.tile([H - 128, C], f32)
        nc.sync.dma_start(wc2b_s[:], wc2[128:H, :])

        xres = None
        if n_res > 0:
            xres = {
                "l": respool.tile([128, S_res], mdt, name="xres_l", tag="xres_l"),
                "c": respool.tile([128, S_res], mdt, name="xres_c", tag="xres_c"),
            }

        # GpSimd TensorTensor fails walrus codegen ("engine check failed (Pool)") -
        # the offload stays disabled until the toolchain supports it.
        use_gp = False and stats_mode == "reduce" and n_tiles >= 6
        NCH = min(8, n_res) if n_res > 0 else 0  # chunked resident-stats reduces
        GP_MOD = 3  # every 3rd tile's stats go to GpSimd
        dve_tiles = [
            t for t in range(n_tiles) if not (use_gp and t % GP_MOD == GP_MOD - 1)
        ]
        col_of = {t: i for i, t in enumerate(dve_tiles)}
        n_cols = len(dve_tiles) + (4 if use_gp else 0)
        if stats_mode == "reduce":
            # resident region -> NCH chunked reduces; tail tiles -> one col each
            n_cols = NCH + (n_tiles - n_res)
        accbuf = {}
        for key in ("min_l", "max_l", "min_c", "max_c"):
            accbuf[key] = statpool.tile(
                [128, n_cols], f32, name="acc" + key, tag="acc" + key
            )

        def act_copy(dst, src):
            nc.scalar.activation(dst, src, Act.Copy)

        def dve_copy(dst, src):
            nc.vector.tensor_copy(dst, src)

        def load_xt(t, src, which, copy_engine, want_sbuf=True):
            """Feature-major [128, VT] data for voxels [t*VT,(t+1)*VT).

            Returns (sbuf_ap_or_None, psum_ap). When want_sbuf is False and
            the tile is not resident, no SBUF copy is made (caller reads the
            PSUM view directly, e.g. for stats).
            """
            nat = natpool.tile([128, VT], f32, tag="nat" + which)
            nc.sync.dma_start(
                nat[:].rearrange("p (a f) -> p a f", f=128),
                src[t * VT : (t + 1) * VT, :].rearrange("(a p) f -> p a f", p=128),
            )
            ps = pstr.tile([128, VT], f32, tag="pstr")
            for a in range(4):
                nc.tensor.transpose(
                    ps[:, a * 128 : (a + 1) * 128],
                    trv(nat[:, a * 128 : (a + 1) * 128]),
                    trv(ident[:]),
                )
            if t < n_res:
                dst = xres[which][:, t * VT : (t + 1) * VT]
            elif want_sbuf:
                dst = xtpool.tile(
                    [128, VT], mdt, name="xt" + which, tag="xt" + which
                )[:]
            else:
                return None, ps[:], nat[:]
            copy_engine(mmv(dst), ps[:])
            return dst, ps[:], nat[:]

        for _rep in range(reps):
            rctx = ctx.enter_context(ExitStack())
            # ---- pass 1: stats (+ resident retention of transposed input) ----
            gacc = {}
            for t in range(n_tiles):
                for which, src in (("l", lidar), ("c", cam)):
                    dst, ps, nat = load_xt(t, src, which, act_copy, want_sbuf=False)
                    xt = ps
                    if stats_mode == "reduce":
                        if t not in col_of:
                            # GpSimd path: running elementwise min/max over the
                            # natural-layout tiles (finalized after pass 1)
                            for statname, op in (("min", Alu.min), ("max", Alu.max)):
                                key = statname + "_" + which
                                prev = gacc.get(key)
                                new = gaccpool.tile(
                                    [128, VT], f32, name="g" + key, tag="g" + key
                                )
                                if prev is None:
                                    nc.gpsimd.tensor_copy(new[:], nat)
                                else:
                                    nc.gpsimd.tensor_tensor(
                                        out=new[:], in0=prev, in1=nat, op=op
                                    )
                                gacc[key] = new[:]
                            continue
                        if t < n_res:
                            continue  # resident: chunked reduces after the loop
                        c = NCH + (t - n_res)
                        nc.vector.tensor_reduce(
                            accbuf["min_" + which][:, c : c + 1],
                            xt,
                            axis=mybir.AxisListType.X,
                            op=Alu.min,
                        )
                        nc.vector.tensor_reduce(
                            accbuf["max_" + which][:, c : c + 1],
                            xt,
                            axis=mybir.AxisListType.X,
                            op=Alu.max,
                        )
                        continue
                    if stats_mode == "ttr_pair":
                        xr = xt.rearrange("p (v two) -> p two v", two=2)
                        in0, in1 = xr[:, 0, :], xr[:, 1, :]
                    else:  # ttr_contig
                        in0, in1 = xt[:, : VT // 2], xt[:, VT // 2 :]
                    j0 = junkpool.tile([128, VT // 2], f32, tag="junk")
                    nc.vector.tensor_tensor_reduce(
                        out=j0[:],
                        in0=in0,
                        in1=in1,
                        scale=1.0,
                        scalar=FMAX,
                        op0=Alu.min,
                        op1=Alu.min,
                        accum_out=accbuf["min_" + which][:, t : t + 1],
                    )
                    j1 = junkpool.tile([128, VT // 2], f32, tag="junk")
                    nc.vector.tensor_tensor_reduce(
                        out=j1[:],
                        in0=in0,
                        in1=in1,
                        scale=1.0,
                        scalar=-FMAX,
                        op0=Alu.max,
                        op1=Alu.max,
                        accum_out=accbuf["max_" + which][:, t : t + 1],
                    )

            if stats_mode == "reduce" and NCH > 0:
                spans = [
                    (n_res * i // NCH, n_res * (i + 1) // NCH) for i in range(NCH)
                ]
                for which in ("l", "c"):
                    for i, (t0s, t1s) in enumerate(spans):
                        span = xres[which][:, t0s * VT : t1s * VT]
                        nc.vector.tensor_reduce(
                            accbuf["min_" + which][:, i : i + 1],
                            span,
                            axis=mybir.AxisListType.X,
                            op=Alu.min,
                        )
                        nc.vector.tensor_reduce(
                            accbuf["max_" + which][:, i : i + 1],
                            span,
                            axis=mybir.AxisListType.X,
                            op=Alu.max,
                        )

            # ---- pass 1.5: finalize stats, tiny attention MLPs, fold gating ----
            if stats_mode == "reduce" and use_gp:
                cb = len(dve_tiles)
                with tc.tile_pool(name="psfin", bufs=2, space="PSUM") as psfin:
                    for key in ("min_l", "max_l", "min_c", "max_c"):
                        op = Alu.min if key.startswith("min") else Alu.max
                        acc = gacc[key]
                        for a in range(4):
                            pst = psfin.tile(
                                [128, 128], f32, name="psfin", tag="psfin"
                            )
                            nc.tensor.transpose(
                                pst[:], acc[:, a * 128 : (a + 1) * 128], ident[:]
                            )
                            nc.vector.tensor_reduce(
                                accbuf[key][:, cb + a : cb + a + 1],
                                pst[:],
                                axis=mybir.AxisListType.X,
                                op=op,
                            )
            stat = {}
            for key in ("min_l", "max_l", "min_c", "max_c"):
                s = statpool.tile([128, 1], f32, tag="stat" + key)
                nc.vector.tensor_reduce(
                    s[:],
                    accbuf[key][:],
                    axis=mybir.AxisListType.X,
                    op=Alu.min if key.startswith("min") else Alu.max,
                )
                stat[key] = s
            cat_chunks = [stat["min_l"], stat["max_l"], stat["min_c"], stat["max_c"]]

            with tc.tile_pool(name="pstiny", bufs=1, space="PSUM") as pstiny:

                def tiny_mlp(w1_s, w2a_s, w2b_s, name):
                    h1_sb = []
                    for tag, mo, mn in (("h1a", 0, 128), ("h1b", 128, H - 128)):
                        ps = pstiny.tile([mn, 1], f32, tag=tag + name)
                        for k in range(4):
                            nc.tensor.matmul(
                                ps[:],
                                w1_s[:, k, mo : mo + mn],
                                cat_chunks[k][:],
                                start=(k == 0),
                                stop=(k == 3),
                            )
                        hs = statpool.tile([mn, 1], f32, tag=tag + "s" + name)
                        nc.scalar.activation(hs[:], ps[:], Act.Relu)
                        h1_sb.append(hs)
                    att_ps = pstiny.tile([128, 1], f32, tag="attps" + name)
                    nc.tensor.matmul(att_ps[:], w2a_s[:], h1_sb[0][:], start=True, stop=False)
                    nc.tensor.matmul(att_ps[:], w2b_s[:], h1_sb[1][:], start=False, stop=True)
                    att_r = statpool.tile([128, 1], f32, tag="attr" + name)
                    nc.scalar.activation(att_r[:], att_ps[:], Act.Relu)
                    att = statpool.tile([128, 1], f32, tag="att" + name)
                    nc.scalar.activation(att[:], att_r[:], Act.Sigmoid)
                    return att

                att_l = tiny_mlp(wl1_s, wl2a_s, wl2b_s, "l")
                att_c = tiny_mlp(wc1_s, wc2a_s, wc2b_s, "c")

            nc.vector.tensor_scalar(
                out=mmv(w1e_s[:, 0, :]), in0=wf1_s[:, 0, :], scalar1=att_l[:],
                scalar2=None, op0=Alu.mult,
            )
            nc.vector.tensor_scalar(
                out=mmv(w1e_s[:, 1, :]), in0=wf1_s[:, 1, :], scalar1=att_c[:],
                scalar2=None, op0=Alu.mult,
            )

            # ---- pass 2: big gated MLP ----
            psl1 = rctx.enter_context(tc.tile_pool(name="psl1", bufs=4, space="PSUM"))
            psl2 = rctx.enter_context(tc.tile_pool(name="psl2", bufs=2, space="PSUM"))
            h1pool = rctx.enter_context(tc.tile_pool(name="h1", bufs=2))
            outpool = rctx.enter_context(tc.tile_pool(name="outp", bufs=4))

            for t in range(n_tiles):
                if t < n_res:
                    xt_l = xres["l"][:, t * VT : (t + 1) * VT]
                    xt_c = xres["c"][:, t * VT : (t + 1) * VT]
                else:
                    xt_l, _, _ = load_xt(t, lidar, "l", dve_copy)
                    xt_c, _, _ = load_xt(t, cam, "c", act_copy)
                h1t = []
                for m in range(2):
                    ps = psl1.tile([128, VT], f32, tag="psl1")
                    nc.tensor.matmul(
                        ps[:], mmv(w1e_s[:, 0, m * 128 : (m + 1) * 128]), mmv(xt_l),
                        start=True, stop=False,
                    )
                    nc.tensor.matmul(
                        ps[:], mmv(w1e_s[:, 1, m * 128 : (m + 1) * 128]), mmv(xt_c),
                        start=False, stop=True,
                    )
                    h = h1pool.tile([128, VT], mdt, tag="h1_%d" % m)
                    if m == 0:
                        nc.vector.tensor_scalar_max(mmv(h[:]), ps[:], 0.0)
                    else:
                        nc.scalar.activation(mmv(h[:]), ps[:], Act.Relu)
                    h1t.append(h)
                for g in range(2):
                    ps2 = psl2.tile([128, 2 * OUT], f32, tag="psl2")
                    for h in range(2):
                        v = g * 2 + h
                        nc.tensor.matmul(
                            ps2[:, h * OUT : (h + 1) * OUT],
                            mmv(h1t[0][:, v * 128 : (v + 1) * 128]),
                            mmv(wf2_s[:, 0, :]),
                            start=True, stop=False,
                        )
                        nc.tensor.matmul(
                            ps2[:, h * OUT : (h + 1) * OUT],
                            mmv(h1t[1][:, v * 128 : (v + 1) * 128]),
                            mmv(wf2_s[:, 1, :]),
                            start=False, stop=True,
                        )
                    ob = outpool.tile([128, 2 * OUT], f32, tag="ob")
                    if g == 0:
                        nc.scalar.activation(ob[:], ps2[:], Act.Relu)
                    else:
                        nc.vector.tensor_scalar_max(ob[:], ps2[:], 0.0)
                    r0 = t * VT + g * 256
                    nc.sync.dma_start(
                        out[r0 : r0 + 256, :].rearrange("(a p) f -> p a f", p=128),
                        ob[:].rearrange("p (a f) -> p a f", f=OUT),
                    )

            rctx.close()

    nc.compile()
    return nc


def _get_program(S_pad):
    cap = S_RES_CAP * (2 if MM_DT == "bf16" else 1)
    key = (S_pad, MM_DT, TR_F32R, cap, STATS_MODE)
    if key not in _cache:
        S_res = min(S_pad, cap - cap % VT)
        _cache[key] = _build(S_pad, S_res, MM_DT, TR_F32R, STATS_MODE)
    return _cache[key]


def shard_inputs(lidar, cam, batch_idx, W_l1, W_l2, W_c1, W_c2, W_f1, W_f2):
    """Split by batch (batch_idx sorted), pad with replicated real rows."""
    lidar = np.ascontiguousarray(lidar, dtype=np.float32)
    cam = np.ascontiguousarray(cam, dtype=np.float32)
    batch_idx = np.asarray(batch_idx)
    bounds = np.searchsorted(batch_idx, np.arange(B + 1))
    sizes = np.diff(bounds)
    S_pad = int(-(-max(int(sizes.max()), 1) // VT) * VT)
    weights = {
        "W_l1": np.ascontiguousarray(W_l1, np.float32),
        "W_l2": np.ascontiguousarray(W_l2, np.float32),
        "W_c1": np.ascontiguousarray(W_c1, np.float32),
        "W_c2": np.ascontiguousarray(W_c2, np.float32),
        "W_f1": np.ascontiguousarray(W_f1, np.float32),
        "W_f2": np.ascontiguousarray(W_f2, np.float32),
    }
    in_maps = []
    for b in range(B):
        s0, s1 = int(bounds[b]), int(bounds[b + 1])
        n = s1 - s0
        l = np.empty((S_pad, L), np.float32)
        c = np.empty((S_pad, C), np.float32)
        if n > 0:
            l[:n] = lidar[s0:s1]
            c[:n] = cam[s0:s1]
            l[n:] = lidar[s1 - 1]
            c[n:] = cam[s1 - 1]
        else:
            l[:] = 0.0
            c[:] = 0.0
        in_maps.append({"lidar": l, "cam": c, **weights})
    return in_maps, bounds, sizes, S_pad


def kernel(lidar, cam, batch_idx, W_l1, W_l2, W_c1, W_c2, W_f1, W_f2):
    from concourse.bass_utils import run_bass_kernel_spmd

    in_maps, bounds, sizes, S_pad = shard_inputs(
        lidar, cam, batch_idx, W_l1, W_l2, W_c1, W_c2, W_f1, W_f2
    )
    nc = _get_program(S_pad)
    res = run_bass_kernel_spmd(nc, in_maps, core_ids=list(range(B)))
    N = lidar.shape[0]
    out_full = np.empty((N, OUT), np.float32)
    for b in range(B):
        s0, s1 = int(bounds[b]), int(bounds[b + 1])
        if s1 > s0:
            out_full[s0:s1] = res.results[b]["out"][: s1 - s0]
    return out_full

